# revision 38
# baseline (speedup 1.0000x reference)
"""AttnBlock (GroupNorm -> qkv 1x1 -> softmax attention -> proj -> residual)
for x (2, 512, 64, 64) on 8 Trainium2 NeuronCores.

Sharding: core i handles batch i//4 and query-token block i%4 (1024 of 4096
spatial tokens). k/v are computed per-core over all 4096 tokens (no
collectives). Inputs are token-rolled per core so every core runs the same
SPMD graph with its own query block at token offset 0 (attention is
permutation-invariant over key/value tokens).

GroupNorm is folded into the projections: hn_c = a_c * x_c + d_c with
a_c = gamma_c * rsqrt(var_g + eps), d_c = beta_c - a_c * mu_g, so
q = (Wq diag(a)) x + (Wq d + bq), etc. The attention scale 1/sqrt(C) is
folded into Wq/bq. k's bias is dropped entirely (a per-row constant in the
scores, cancelled by softmax); v's bias is folded into the output bias
(bo' = bo + Wo bv) because sum_j softmax = 1; bo' is pre-added into the
residual tile. Softmax runs without max subtraction (normalized inputs bound
scores ~N(0,1); exp cannot overflow f32), so ScalarE exponentiates score
chunks straight out of PSUM while accumulating the denominators. The
(i,j)->(j,i) attention transpose and the (i,c)->(c,i) output transpose run
on the DMA XBAR (2-byte), keeping TensorE and the DVE out of it.
"""

import numpy as np

C = 512          # channels
N = 4096         # spatial tokens (64*64)
NB = 1024        # query tokens per core
G = 32           # groups
CT = 4           # channel tiles of 128
EPS = 1e-6
SCALE = float(C) ** -0.5
QKSCALE = float(C) ** -0.25  # split between q and k so fp8 sees a good range
NCORES = 8

_cache = {}


def _split_sync_waits(nc, maxw=1):
    """This walrus build encodes at most ~1 sync wait per instruction
    descriptor. Move excess sem waits onto same-engine nops inserted just
    before the instruction (in-order sequencers make this equivalent)."""
    from concourse import mybir

    n = 0
    for fn in nc.m.functions:
        for b in fn.blocks:
            out = []
            for ins in b.instructions:
                si = getattr(ins, "sync_info", None)
                if si is not None and si.on_wait and len(si.on_wait) > maxw:
                    waits = list(si.on_wait)
                    extra, keep = waits[:-maxw], waits[-maxw:]
                    for j in range(0, len(extra), maxw):
                        nop = mybir.InstNoOp(name=f"I-wsp{n}", ins=[], outs=[])
                        n += 1
                        nop.engine = ins.engine
                        nop.sync_info = mybir.SyncInfo(
                            on_wait=extra[j : j + maxw], on_update=[]
                        )
                        out.append(nop)
                    ins.sync_info = mybir.SyncInfo(
                        on_wait=keep, on_update=list(si.on_update)
                    )
                out.append(ins)
            b.instructions = out


def build(split_waits=True):
    import concourse.bass as bass
    import concourse.tile as tile
    from concourse import mybir

    f32 = mybir.dt.float32
    bf16 = mybir.dt.bfloat16
    fp8 = mybir.dt.float8e4
    AX = mybir.AxisListType
    ALU = mybir.AluOpType
    ACT = mybir.ActivationFunctionType
    DROW = mybir.MatmulPerfMode.DoubleRow

    nc = bass.Bass()
    X = nc.declare_dram_parameter("x", [CT, 128, N], f32, isOutput=False)
    XB = nc.declare_dram_parameter("x_bf", [CT, 128, N], bf16, isOutput=False)
    WQ = nc.declare_dram_parameter("wq_t", [C, C], bf16, isOutput=False)
    WK = nc.declare_dram_parameter("wk_t", [C, C], bf16, isOutput=False)
    WV = nc.declare_dram_parameter("wv_t", [C, C], bf16, isOutput=False)
    WO = nc.declare_dram_parameter("wo_t", [C, C], bf16, isOutput=False)
    GAM = nc.declare_dram_parameter("gamma", [128, CT, 1], f32, isOutput=False)
    BET = nc.declare_dram_parameter("beta", [128, CT, 1], f32, isOutput=False)
    BQS = nc.declare_dram_parameter("bq_s", [128, CT, 1], f32, isOutput=False)
    BV = nc.declare_dram_parameter("bv", [128, CT, 1], bf16, isOutput=False)
    BO = nc.declare_dram_parameter("bo", [128, CT, 1], f32, isOutput=False)
    GS = nc.declare_dram_parameter("gsum", [128, CT, G], f32, isOutput=False)
    GB = nc.declare_dram_parameter("gbcast", [G, CT, 128], f32, isOutput=False)
    OUT = nc.declare_dram_parameter("out", [CT, 128, NB], f32, isOutput=True)
    OUT_p = OUT.rearrange("a p t -> p a t")

    w_re = {
        "q": WQ.rearrange("(a p) o -> p a o", p=128),
        "k": WK.rearrange("(a p) o -> p a o", p=128),
        "v": WV.rearrange("(a p) o -> p a o", p=128),
        "o": WO.rearrange("(a p) o -> p a o", p=128),
    }

    with tile.TileContext(nc) as tc:
        with (
            tc.tile_pool(name="singles", bufs=1) as singles,
            tc.tile_pool(name="persist", bufs=1) as persist,
            tc.tile_pool(name="ps_big", bufs=2, space="PSUM") as ps_big,
        ):
            # persistent attention tensors
            xres = persist.tile([128, CT, NB], f32)
            wq_s = persist.tile([128, CT, C], bf16)
            wk_s = persist.tile([128, CT, C], bf16)
            wv_s = persist.tile([128, CT, C], bf16)
            wo_b = persist.tile([128, CT, C], bf16)
            bias_o = persist.tile([128, CT, 1], f32)
            q_sb = persist.tile([128, CT, NB], fp8)
            k_sb = persist.tile([128, CT, N], fp8)
            vT_sb = persist.tile([128, N // 128, C], bf16)

            with (
                tc.tile_pool(name="xbp", bufs=1) as xbp,
                tc.tile_pool(name="wfp", bufs=2) as wfp,
                tc.tile_pool(name="statp", bufs=2) as statp,
                tc.tile_pool(name="ps_st", bufs=2, space="PSUM") as ps_st,
            ):
                # ---- x: host-prepared bf16 copy; raw moments chase the DMA ----
                xb = xbp.tile([128, CT, N], bf16)
                H = 256
                for ct in range(CT):
                    for h in range(N // H):
                        nc.sync.dma_start(
                            out=xb[:, ct, h * H : (h + 1) * H],
                            in_=XB[ct, :, h * H : (h + 1) * H],
                        )
                # per-1024-token partial moments, spread over three engines:
                # sums[:, ct, h] = sum_t x (part h); sums[:, ct, 4+h] = sum x^2
                NH = 4
                HW = N // NH
                sums = statp.tile([128, CT, 2 * NH], f32, tag="sums")
                for ct in range(CT):
                    for h in range(NH):
                        nc.vector.reduce_sum(
                            out=sums[:, ct, h : h + 1],
                            in_=xb[:, ct, h * HW : (h + 1) * HW],
                            axis=AX.X,
                        )
                        xc = xb[:, ct, h * HW : (h + 1) * HW]
                        if ct < 3:
                            junk = statp.tile([128, HW], bf16, tag="junk")
                            nc.scalar.activation(
                                out=junk,
                                in_=xc,
                                func=ACT.Square,
                                accum_out=sums[:, ct, NH + h : NH + h + 1],
                            )
                        else:
                            junk = statp.tile([128, HW], f32, tag="junkg")
                            nc.gpsimd.tensor_mul(out=junk, in0=xc, in1=xc)
                            nc.vector.reduce_sum(
                                out=sums[:, ct, NH + h : NH + h + 1],
                                in_=junk,
                                axis=AX.X,
                            )

                # ---- constants / small inputs ----
                gam_t = singles.tile([128, CT, 1], f32)
                nc.sync.dma_start(out=gam_t, in_=GAM[:, :, :])
                bet_t = singles.tile([128, CT, 1], f32)
                nc.sync.dma_start(out=bet_t, in_=BET[:, :, :])
                bqs_t = singles.tile([128, CT, 1], f32)
                nc.sync.dma_start(out=bqs_t, in_=BQS[:, :, :])
                bv_t = singles.tile([128, CT, 1], bf16)
                nc.sync.dma_start(out=bv_t, in_=BV[:, :, :])
                bo_t = singles.tile([128, CT, 1], f32)
                nc.sync.dma_start(out=bo_t, in_=BO[:, :, :])
                gs_t = singles.tile([128, CT, G], f32)
                nc.sync.dma_start(out=gs_t, in_=GS[:, :, :])
                gb_t = singles.tile([G, CT, 128], f32)
                nc.sync.dma_start(out=gb_t, in_=GB[:, :, :])
                nc.sync.dma_start(out=wo_b, in_=w_re["o"])

                # group sums via indicator matmul on the partials: (32, 2*NH)
                psg = ps_st.tile([128, 2 * NH], f32, tag="ps_small")
                for ct in range(CT):
                    nc.tensor.matmul(
                        psg[:G, :],
                        gs_t[:, ct, :],
                        sums[:, ct, :],
                        start=(ct == 0),
                        stop=(ct == CT - 1),
                    )
                gst = statp.tile([G, 2], f32, tag="gst")
                nc.vector.reduce_sum(out=gst[:, 0:1], in_=psg[:G, 0:NH], axis=AX.X)
                nc.vector.reduce_sum(
                    out=gst[:, 1:2], in_=psg[:G, NH : 2 * NH], axis=AX.X
                )
                nc.scalar.mul(out=gst, in_=gst, mul=1.0 / (16.0 * N))  # [mu_g, E2_g]
                gvar = statp.tile([G, 1], f32, tag="gvar")
                nc.vector.tensor_mul(out=gvar, in0=gst[:, 0:1], in1=gst[:, 0:1])
                nc.vector.tensor_sub(out=gvar, in0=gst[:, 1:2], in1=gvar)
                eps_t = statp.tile([G, 1], f32, tag="eps")
                nc.vector.memset(eps_t, EPS)
                gsq = statp.tile([G, 1], f32, tag="gsq")
                nc.scalar.activation(
                    out=gsq, in_=gvar, func=ACT.Sqrt, bias=eps_t, scale=1.0
                )
                gstat2 = statp.tile([G, 2], f32, tag="gstat2")
                nc.vector.reciprocal(out=gstat2[:, 1:2], in_=gsq)
                nc.vector.tensor_copy(out=gstat2[:, 0:1], in_=gst[:, 0:1])

                # broadcast groups -> channels: mu_inv (128, CT, 2)
                mu_inv = statp.tile([128, CT, 2], f32, tag="mu_inv")
                for ct in range(CT):
                    psb = ps_st.tile([128, 2], f32, tag="ps_small")
                    nc.tensor.matmul(
                        psb, gb_t[:, ct, :], gstat2, start=True, stop=True
                    )
                    nc.vector.tensor_copy(out=mu_inv[:, ct, :], in_=psb)

                # a = gamma * inv ; d = beta - a * mu ; aq = a * SCALE
                a_t = statp.tile([128, CT, 1], f32, tag="a_t")
                nc.vector.tensor_mul(out=a_t, in0=gam_t, in1=mu_inv[:, :, 1:2])
                d_t = statp.tile([128, CT, 1], f32, tag="d_t")
                nc.vector.tensor_mul(out=d_t, in0=a_t, in1=mu_inv[:, :, 0:1])
                nc.vector.tensor_sub(out=d_t, in0=bet_t, in1=d_t)
                aq_t = statp.tile([128, CT, 1], f32, tag="aq_t")
                nc.scalar.mul(out=aq_t, in_=a_t, mul=QKSCALE)
                d_bf = statp.tile([128, CT, 1], bf16, tag="d_bf")
                nc.vector.tensor_copy(out=d_bf, in_=d_t)

                # bias_o = Wo bv + bo
                for ot in range(CT):
                    pb = ps_st.tile([128, 2], f32, tag="ps_small")
                    for ct in range(CT):
                        nc.tensor.matmul(
                            pb[:, 0:1],
                            wo_b[:, ct, ot * 128 : (ot + 1) * 128],
                            bv_t[:, ct, :],
                            start=(ct == 0),
                            stop=(ct == CT - 1),
                        )
                    nc.scalar.activation(
                        out=bias_o[:, ot, :],
                        in_=pb[:, 0:1],
                        func=ACT.Identity,
                        bias=bo_t[:, ot, :],
                        scale=1.0,
                    )

                # stream q/k/v weights: fold + q bias projection
                bias_q = statp.tile([128, CT, 1], f32, tag="bias_q")
                for wname, wdst, scal, bvec, bdst, bscale in (
                    ("q", wq_s, aq_t, d_bf, bias_q, QKSCALE),
                    ("k", wk_s, aq_t, None, None, None),
                    ("v", wv_s, a_t, None, None, None),
                ):
                    wf = wfp.tile([128, CT, C], bf16, tag="wf")
                    nc.sync.dma_start(out=wf, in_=w_re[wname])
                    for ct in range(CT):
                        nc.vector.tensor_scalar_mul(
                            out=wdst[:, ct, :],
                            in0=wf[:, ct, :],
                            scalar1=scal[:, ct, :],
                        )
                    if bvec is not None:
                        for ot in range(CT):
                            pb = ps_st.tile([128, 2], f32, tag="ps_small")
                            for ct in range(CT):
                                nc.tensor.matmul(
                                    pb[:, 0:1],
                                    wf[:, ct, ot * 128 : (ot + 1) * 128],
                                    bvec[:, ct, :],
                                    start=(ct == 0),
                                    stop=(ct == CT - 1),
                                )
                            nc.scalar.activation(
                                out=bdst[:, ot, :],
                                in_=pb[:, 0:1],
                                func=ACT.Identity,
                                bias=bqs_t[:, ot, :],
                                scale=bscale,
                            )

                # ---- projections ----
                for ot in range(CT):
                    for jc in range(NB // 512):
                        ps = ps_big.tile([128, 512], f32, tag="psbig")
                        for ct in range(CT):
                            nc.tensor.matmul(
                                ps,
                                wq_s[:, ct, ot * 128 : (ot + 1) * 128],
                                xb[:, ct, jc * 512 : (jc + 1) * 512],
                                start=(ct == 0),
                                stop=(ct == CT - 1),
                            )
                        nc.scalar.activation(
                            out=q_sb[:, ot, jc * 512 : (jc + 1) * 512],
                            in_=ps,
                            func=ACT.Identity,
                            bias=bias_q[:, ot, :],
                            scale=1.0,
                        )

                for ot in range(CT):
                    for jc in range(N // 512):
                        ps = ps_big.tile([128, 512], f32, tag="psbig")
                        for ct in range(CT):
                            nc.tensor.matmul(
                                ps,
                                wk_s[:, ct, ot * 128 : (ot + 1) * 128],
                                xb[:, ct, jc * 512 : (jc + 1) * 512],
                                start=(ct == 0),
                                stop=(ct == CT - 1),
                            )
                        dst = k_sb[:, ot, jc * 512 : (jc + 1) * 512]
                        if jc % 8 < 5:
                            nc.vector.tensor_copy(out=dst, in_=ps)
                        else:
                            nc.scalar.activation(out=dst, in_=ps, func=ACT.Copy)

                for tb in range(N // 128):
                    ps = ps_big.tile([128, 512], f32, tag="psbig")
                    for ct in range(CT):
                        nc.tensor.matmul(
                            ps,
                            xb[:, ct, tb * 128 : (tb + 1) * 128],
                            wv_s[:, ct, :],
                            start=(ct == 0),
                            stop=(ct == CT - 1),
                        )
                    if tb % 8 < 5:
                        nc.vector.tensor_copy(out=vT_sb[:, tb, :], in_=ps)
                    else:
                        nc.scalar.activation(
                            out=vT_sb[:, tb, :], in_=ps, func=ACT.Copy
                        )

                # residual slice (full channels, my 1024 tokens); loaded late
                # so it doesn't compete with xb/weights for head DMA
                nc.sync.dma_start(
                    out=xres, in_=X.rearrange("a p t -> p a t")[:, :, 0:NB]
                )
                # fold the output bias into the residual tile: the epilogue
                # then needs a single add per query block
                for ot in range(CT):
                    nc.vector.tensor_scalar_add(
                        out=xres[:, ot, :],
                        in0=xres[:, ot, :],
                        scalar1=bias_o[:, ot, :],
                    )

            # ---- attention over 8 query blocks of 128 ----
            with (
                tc.tile_pool(name="loopp", bufs=2) as loopp,
                tc.tile_pool(name="sblk", bufs=2) as sblk,
                tc.tile_pool(name="ps_av", bufs=2, space="PSUM") as ps_av,
                tc.tile_pool(name="ps_o", bufs=2, space="PSUM") as ps_o,
            ):
                for ib in range(NB // 128):
                    i0 = ib * 128
                    p_sb = sblk.tile([128, N], bf16, tag="p_sb")
                    denp = loopp.tile([128, 4], f32, tag="denp")
                    for jc in range(N // 1024):
                        ps = ps_big.tile([128, 1024], f32, tag="psbig")
                        for half in range(2):
                            for c2 in range(2):
                                nc.tensor.matmul(
                                    ps[:, half * 512 : (half + 1) * 512],
                                    q_sb[:, 2 * c2 : 2 * c2 + 2, i0 : i0 + 128],
                                    k_sb[
                                        :,
                                        2 * c2 : 2 * c2 + 2,
                                        jc * 1024
                                        + half * 512 : jc * 1024
                                        + (half + 1) * 512,
                                    ],
                                    start=(c2 == 0),
                                    stop=(c2 == 1),
                                    perf_mode=DROW,
                                )
                        nc.scalar.activation(
                            out=p_sb[:, jc * 1024 : (jc + 1) * 1024],
                            in_=ps,
                            func=ACT.Exp,
                            accum_out=denp[:, jc : jc + 1],
                        )
                    den = loopp.tile([128, 1], f32, tag="den")
                    nc.vector.reduce_sum(out=den, in_=denp, axis=AX.X)
                    rden = loopp.tile([128, 1], f32, tag="rden")
                    nc.vector.reciprocal(out=rden, in_=den)

                    pT_sb = sblk.tile([128, N // 128, 128], bf16, tag="pT_sb")
                    nc.sync.dma_start_transpose(pT_sb, p_sb)

                    pav = ps_av.tile([128, C], f32, tag="pav")
                    for jb in range(N // 128):
                        nc.tensor.matmul(
                            pav,
                            pT_sb[:, jb, :],
                            vT_sb[:, jb, :],
                            start=(jb == 0),
                            stop=(jb == N // 128 - 1),
                        )
                    o2 = loopp.tile([128, C], bf16, tag="o2")
                    nc.vector.tensor_scalar_mul(out=o2, in0=pav, scalar1=rden)

                    ao = loopp.tile([128, CT, 128], bf16, tag="ao")
                    nc.sync.dma_start_transpose(ao, o2)

                    outf = loopp.tile([128, CT, 128], f32, tag="outf")
                    pso = ps_o.tile([128, CT, 128], f32, tag="pso")
                    for ot in range(CT):
                        for ct in range(CT):
                            nc.tensor.matmul(
                                pso[:, ot, :],
                                wo_b[:, ct, ot * 128 : (ot + 1) * 128],
                                ao[:, ct, :],
                                start=(ct == 0),
                                stop=(ct == CT - 1),
                            )
                    nc.vector.tensor_add(
                        out=outf, in0=pso, in1=xres[:, :, i0 : i0 + 128]
                    )
                    nc.sync.dma_start(out=OUT_p[:, :, i0 : i0 + 128], in_=outf)

    if split_waits:
        _split_sync_waits(nc)
    return nc


def _prep_in_maps(x, gn_gamma, gn_beta, wq, bq, wk, bk, wv, bv, wo, bo):
    import ml_dtypes

    f = np.float32
    bf = ml_dtypes.bfloat16
    xr = np.asarray(x, f).reshape(2, C, N)
    wq_t = np.ascontiguousarray(np.asarray(wq, f).T.astype(bf))
    wk_t = np.ascontiguousarray(np.asarray(wk, f).T.astype(bf))
    wv_t = np.ascontiguousarray(np.asarray(wv, f).T.astype(bf))
    wo_t = np.ascontiguousarray(np.asarray(wo, f).T.astype(bf))

    def vec(v, dt=f):
        return np.ascontiguousarray(
            np.asarray(v, f).reshape(CT, 128).transpose(1, 0)[:, :, None].astype(dt)
        )

    gam = vec(gn_gamma)
    bet = vec(gn_beta)
    bq_s = vec(np.asarray(bq, f) * QKSCALE)
    bvv = vec(bv, bf)
    bov = vec(bo)

    cidx = np.arange(C)
    grp = cidx // 16  # (512,)
    gsum = np.zeros((128, CT, G), f)
    gbcast = np.zeros((G, CT, 128), f)
    for ct in range(CT):
        for cl in range(128):
            g = grp[ct * 128 + cl]
            gsum[cl, ct, g] = 1.0
            gbcast[g, ct, cl] = 1.0

    in_maps = []
    for core in range(NCORES):
        b, r = divmod(core, 4)
        xroll = np.ascontiguousarray(np.roll(xr[b], -r * NB, axis=1).reshape(CT, 128, N))
        in_maps.append(
            {
                "x": xroll,
                "x_bf": xroll.astype(bf),
                "wq_t": wq_t,
                "wk_t": wk_t,
                "wv_t": wv_t,
                "wo_t": wo_t,
                "gamma": gam,
                "beta": bet,
                "bq_s": bq_s,
                "bv": bvv,
                "bo": bov,
                "gsum": gsum,
                "gbcast": gbcast,
            }
        )
    return in_maps


def _assemble(results):
    out = np.empty((2, C, N), np.float32)
    for core in range(NCORES):
        b, r = divmod(core, 4)
        out[b][:, r * NB : (r + 1) * NB] = np.asarray(results[core]["out"]).reshape(
            C, NB
        )
    return out.reshape(2, C, 64, 64)


def _run(in_maps, trace=False, trace_kwargs=None):
    from concourse.bass_utils import run_bass_kernel_spmd

    if "nc" not in _cache:
        _cache["nc"] = build()
    kw = {}
    if trace:
        kw = {"trace": True, "trace_kwargs": trace_kwargs or {}}
    return run_bass_kernel_spmd(
        _cache["nc"], in_maps, core_ids=list(range(NCORES)), **kw
    )


def kernel(x, gn_gamma, gn_beta, wq, bq, wk, bk, wv, bv, wo, bo):
    in_maps = _prep_in_maps(x, gn_gamma, gn_beta, wq, bq, wk, bk, wv, bv, wo, bo)
    res = _run(in_maps, trace=False)
    return _assemble(res.results)


# revision 47
# speedup vs baseline: 1.1185x; 1.1185x over previous
"""AttnBlock (GroupNorm -> qkv 1x1 -> softmax attention -> proj -> residual)
for x (2, 512, 64, 64) on 8 Trainium2 NeuronCores.

Sharding: core i handles batch i//4 and query-token block i%4 (1024 of 4096
spatial tokens). k/v are computed per-core over all 4096 tokens (no
collectives). Inputs are token-rolled per core so every core runs the same
SPMD graph with its own query block at token offset 0 (attention is
permutation-invariant over key/value tokens).

GroupNorm is folded into the projections: hn_c = a_c * x_c + d_c with
a_c = gamma_c * rsqrt(var_g + eps), d_c = beta_c - a_c * mu_g, so
q = (Wq diag(a)) x + (Wq d + bq), etc. The attention scale 1/sqrt(C) is
folded into Wq/bq. k's bias is dropped entirely (a per-row constant in the
scores, cancelled by softmax); v's bias is folded into the output bias
(bo' = bo + Wo bv) because sum_j softmax = 1; bo' is pre-added into the
residual tile. Softmax runs without max subtraction (normalized inputs bound
scores ~N(0,1); exp cannot overflow f32), so ScalarE exponentiates score
chunks straight out of PSUM while accumulating the denominators. The
(i,j)->(j,i) attention transpose and the (i,c)->(c,i) output transpose run
on the DMA XBAR (2-byte), keeping TensorE and the DVE out of it.
"""

import numpy as np

C = 512          # channels
N = 4096         # spatial tokens (64*64)
NB = 1024        # query tokens per core
G = 32           # groups
CT = 4           # channel tiles of 128
EPS = 1e-6
SCALE = float(C) ** -0.5
QKSCALE = float(C) ** -0.25  # split between q and k so fp8 sees a good range
NCORES = 8

_cache = {}


def _split_sync_waits(nc, maxw=1):
    """This walrus build encodes at most ~1 sync wait per instruction
    descriptor. Move excess sem waits onto same-engine nops inserted just
    before the instruction (in-order sequencers make this equivalent)."""
    from concourse import mybir

    n = 0
    for fn in nc.m.functions:
        for b in fn.blocks:
            out = []
            for ins in b.instructions:
                si = getattr(ins, "sync_info", None)
                if si is not None and si.on_wait and len(si.on_wait) > maxw:
                    waits = list(si.on_wait)
                    extra, keep = waits[:-maxw], waits[-maxw:]
                    for j in range(0, len(extra), maxw):
                        nop = mybir.InstNoOp(name=f"I-wsp{n}", ins=[], outs=[])
                        n += 1
                        nop.engine = ins.engine
                        nop.sync_info = mybir.SyncInfo(
                            on_wait=extra[j : j + maxw], on_update=[]
                        )
                        out.append(nop)
                    ins.sync_info = mybir.SyncInfo(
                        on_wait=keep, on_update=list(si.on_update)
                    )
                out.append(ins)
            b.instructions = out


def build(split_waits=True):
    import concourse.bass as bass
    import concourse.tile as tile
    from concourse import mybir

    f32 = mybir.dt.float32
    bf16 = mybir.dt.bfloat16
    fp8 = mybir.dt.float8e4
    AX = mybir.AxisListType
    ALU = mybir.AluOpType
    ACT = mybir.ActivationFunctionType
    DROW = mybir.MatmulPerfMode.DoubleRow

    nc = bass.Bass()
    X = nc.declare_dram_parameter("x", [CT, 128, N], f32, isOutput=False)
    XB = nc.declare_dram_parameter("x_bf", [CT, 128, N], bf16, isOutput=False)
    X8 = nc.declare_dram_parameter("x_f8", [CT, 128, N], fp8, isOutput=False)
    WQ = nc.declare_dram_parameter("wq_t", [C, C], bf16, isOutput=False)
    WK = nc.declare_dram_parameter("wk_t", [C, C], bf16, isOutput=False)
    WV = nc.declare_dram_parameter("wv_t", [C, C], bf16, isOutput=False)
    WO = nc.declare_dram_parameter("wo_t", [C, C], bf16, isOutput=False)
    GAM = nc.declare_dram_parameter("gamma", [128, CT, 1], f32, isOutput=False)
    BET = nc.declare_dram_parameter("beta", [128, CT, 1], f32, isOutput=False)
    BQS = nc.declare_dram_parameter("bq_s", [128, CT, 1], f32, isOutput=False)
    BV = nc.declare_dram_parameter("bv", [128, CT, 1], bf16, isOutput=False)
    BO = nc.declare_dram_parameter("bo", [128, CT, 1], f32, isOutput=False)
    GS = nc.declare_dram_parameter("gsum", [128, CT, G], f32, isOutput=False)
    GB = nc.declare_dram_parameter("gbcast", [G, CT, 128], f32, isOutput=False)
    OUT = nc.declare_dram_parameter("out", [CT, 128, NB], f32, isOutput=True)
    OUT_p = OUT.rearrange("a p t -> p a t")

    w_re = {
        "q": WQ.rearrange("(a p) o -> p a o", p=128),
        "k": WK.rearrange("(a p) o -> p a o", p=128),
        "v": WV.rearrange("(a p) o -> p a o", p=128),
        "o": WO.rearrange("(a p) o -> p a o", p=128),
    }

    with tile.TileContext(nc) as tc:
        with (
            tc.tile_pool(name="singles", bufs=1) as singles,
            tc.tile_pool(name="persist", bufs=1) as persist,
            tc.tile_pool(name="ps_big", bufs=2, space="PSUM") as ps_big,
        ):
            # persistent attention tensors
            xres = persist.tile([128, CT, NB], f32)
            wq_s = persist.tile([128, CT, C], fp8)
            wk_s = persist.tile([128, CT, C], fp8)
            wv_s = persist.tile([128, CT, C], bf16)
            wo_b = persist.tile([128, CT, C], bf16)
            bias_o = persist.tile([128, CT, 1], f32)
            q_sb = persist.tile([128, CT, NB], fp8)
            k_sb = persist.tile([128, CT, N], fp8)
            vT_sb = persist.tile([128, N // 128, C], bf16)

            with (
                tc.tile_pool(name="xbp", bufs=1) as xbp,
                tc.tile_pool(name="wfp", bufs=2) as wfp,
                tc.tile_pool(name="statp", bufs=2) as statp,
                tc.tile_pool(name="ps_st", bufs=2, space="PSUM") as ps_st,
            ):
                # ---- x: host-prepared bf16 copy; raw moments chase the DMA ----
                xb = xbp.tile([128, CT, N], bf16)
                H = 512
                for ct in range(CT):
                    for h in range(N // H):
                        nc.sync.dma_start(
                            out=xb[:, ct, h * H : (h + 1) * H],
                            in_=XB[ct, :, h * H : (h + 1) * H],
                        )
                x8 = xbp.tile([128, CT, N], fp8)
                nc.sync.dma_start(out=x8, in_=X8[:, :, :].rearrange("a p t -> p a t"))
                # per-1024-token partial moments, spread over three engines:
                # sums[:, ct, h] = sum_t x (part h); sums[:, ct, 4+h] = sum x^2
                NH = 4
                HW = N // NH
                sums = statp.tile([128, CT, 2 * NH], f32, tag="sums")
                for ct in range(CT):
                    for h in range(NH):
                        nc.vector.reduce_sum(
                            out=sums[:, ct, h : h + 1],
                            in_=xb[:, ct, h * HW : (h + 1) * HW],
                            axis=AX.X,
                        )
                        xc = xb[:, ct, h * HW : (h + 1) * HW]
                        junk = statp.tile([128, HW], bf16, tag="junk")
                        nc.scalar.activation(
                            out=junk,
                            in_=xc,
                            func=ACT.Square,
                            accum_out=sums[:, ct, NH + h : NH + h + 1],
                        )

                # ---- constants / small inputs ----
                gam_t = singles.tile([128, CT, 1], f32)
                nc.sync.dma_start(out=gam_t, in_=GAM[:, :, :])
                bet_t = singles.tile([128, CT, 1], f32)
                nc.sync.dma_start(out=bet_t, in_=BET[:, :, :])
                bqs_t = singles.tile([128, CT, 1], f32)
                nc.sync.dma_start(out=bqs_t, in_=BQS[:, :, :])
                bv_t = singles.tile([128, CT, 1], bf16)
                nc.sync.dma_start(out=bv_t, in_=BV[:, :, :])
                bo_t = singles.tile([128, CT, 1], f32)
                nc.sync.dma_start(out=bo_t, in_=BO[:, :, :])
                gs_t = singles.tile([128, CT, G], f32)
                nc.sync.dma_start(out=gs_t, in_=GS[:, :, :])
                gb_t = singles.tile([G, CT, 128], f32)
                nc.sync.dma_start(out=gb_t, in_=GB[:, :, :])
                nc.sync.dma_start(out=wo_b, in_=w_re["o"])

                # group sums via indicator matmul on the partials: (32, 2*NH)
                psg = ps_st.tile([128, 2 * NH], f32, tag="ps_small")
                for ct in range(CT):
                    nc.tensor.matmul(
                        psg[:G, :],
                        gs_t[:, ct, :],
                        sums[:, ct, :],
                        start=(ct == 0),
                        stop=(ct == CT - 1),
                    )
                gst = statp.tile([G, 2], f32, tag="gst")
                nc.vector.reduce_sum(out=gst[:, 0:1], in_=psg[:G, 0:NH], axis=AX.X)
                nc.vector.reduce_sum(
                    out=gst[:, 1:2], in_=psg[:G, NH : 2 * NH], axis=AX.X
                )
                nc.scalar.mul(out=gst, in_=gst, mul=1.0 / (16.0 * N))  # [mu_g, E2_g]
                gvar = statp.tile([G, 1], f32, tag="gvar")
                nc.vector.tensor_mul(out=gvar, in0=gst[:, 0:1], in1=gst[:, 0:1])
                nc.vector.tensor_sub(out=gvar, in0=gst[:, 1:2], in1=gvar)
                eps_t = statp.tile([G, 1], f32, tag="eps")
                nc.vector.memset(eps_t, EPS)
                gsq = statp.tile([G, 1], f32, tag="gsq")
                nc.scalar.activation(
                    out=gsq, in_=gvar, func=ACT.Sqrt, bias=eps_t, scale=1.0
                )
                gstat2 = statp.tile([G, 2], f32, tag="gstat2")
                nc.vector.reciprocal(out=gstat2[:, 1:2], in_=gsq)
                nc.vector.tensor_copy(out=gstat2[:, 0:1], in_=gst[:, 0:1])

                # broadcast groups -> channels: mu_inv (128, CT, 2)
                mu_inv = statp.tile([128, CT, 2], f32, tag="mu_inv")
                for ct in range(CT):
                    psb = ps_st.tile([128, 2], f32, tag="ps_small")
                    nc.tensor.matmul(
                        psb, gb_t[:, ct, :], gstat2, start=True, stop=True
                    )
                    nc.vector.tensor_copy(out=mu_inv[:, ct, :], in_=psb)

                # a = gamma * inv ; d = beta - a * mu ; aq = a * SCALE
                a_t = statp.tile([128, CT, 1], f32, tag="a_t")
                nc.vector.tensor_mul(out=a_t, in0=gam_t, in1=mu_inv[:, :, 1:2])
                d_t = statp.tile([128, CT, 1], f32, tag="d_t")
                nc.vector.tensor_mul(out=d_t, in0=a_t, in1=mu_inv[:, :, 0:1])
                nc.vector.tensor_sub(out=d_t, in0=bet_t, in1=d_t)
                aq_t = statp.tile([128, CT, 1], f32, tag="aq_t")
                nc.scalar.mul(out=aq_t, in_=a_t, mul=QKSCALE)
                d_bf = statp.tile([128, CT, 1], bf16, tag="d_bf")
                nc.vector.tensor_copy(out=d_bf, in_=d_t)

                # bias_o = Wo bv + bo
                for ot in range(CT):
                    pb = ps_st.tile([128, 2], f32, tag="ps_small")
                    for ct in range(CT):
                        nc.tensor.matmul(
                            pb[:, 0:1],
                            wo_b[:, ct, ot * 128 : (ot + 1) * 128],
                            bv_t[:, ct, :],
                            start=(ct == 0),
                            stop=(ct == CT - 1),
                        )
                    nc.scalar.activation(
                        out=bias_o[:, ot, :],
                        in_=pb[:, 0:1],
                        func=ACT.Identity,
                        bias=bo_t[:, ot, :],
                        scale=1.0,
                    )

                # stream q/k/v weights: fold + q bias projection
                bias_q = statp.tile([128, CT, 1], f32, tag="bias_q")
                for wname, wdst, scal, bvec, bdst, bscale in (
                    ("q", wq_s, aq_t, d_bf, bias_q, QKSCALE),
                    ("k", wk_s, aq_t, None, None, None),
                    ("v", wv_s, a_t, None, None, None),
                ):
                    wf = wfp.tile([128, CT, C], bf16, tag="wf")
                    nc.sync.dma_start(out=wf, in_=w_re[wname])
                    for ct in range(CT):
                        nc.vector.tensor_scalar_mul(
                            out=wdst[:, ct, :],
                            in0=wf[:, ct, :],
                            scalar1=scal[:, ct, :],
                        )
                    if bvec is not None:
                        for ot in range(CT):
                            pb = ps_st.tile([128, 2], f32, tag="ps_small")
                            for ct in range(CT):
                                nc.tensor.matmul(
                                    pb[:, 0:1],
                                    wf[:, ct, ot * 128 : (ot + 1) * 128],
                                    bvec[:, ct, :],
                                    start=(ct == 0),
                                    stop=(ct == CT - 1),
                                )
                            nc.scalar.activation(
                                out=bdst[:, ot, :],
                                in_=pb[:, 0:1],
                                func=ACT.Identity,
                                bias=bqs_t[:, ot, :],
                                scale=bscale,
                            )

                # ---- projections (q/k in fp8 DoubleRow) ----
                for ot in range(CT):
                    for jc in range(NB // 512):
                        ps = ps_big.tile([128, 512], f32, tag="psbig")
                        for c2 in range(2):
                            nc.tensor.matmul(
                                ps,
                                wq_s[:, 2 * c2 : 2 * c2 + 2, ot * 128 : (ot + 1) * 128],
                                x8[:, 2 * c2 : 2 * c2 + 2, jc * 512 : (jc + 1) * 512],
                                start=(c2 == 0),
                                stop=(c2 == 1),
                                perf_mode=DROW,
                            )
                        nc.scalar.activation(
                            out=q_sb[:, ot, jc * 512 : (jc + 1) * 512],
                            in_=ps,
                            func=ACT.Identity,
                            bias=bias_q[:, ot, :],
                            scale=1.0,
                        )

                for ot in range(CT):
                    for jc in range(N // 512):
                        ps = ps_big.tile([128, 512], f32, tag="psbig")
                        for c2 in range(2):
                            nc.tensor.matmul(
                                ps,
                                wk_s[:, 2 * c2 : 2 * c2 + 2, ot * 128 : (ot + 1) * 128],
                                x8[:, 2 * c2 : 2 * c2 + 2, jc * 512 : (jc + 1) * 512],
                                start=(c2 == 0),
                                stop=(c2 == 1),
                                perf_mode=DROW,
                            )
                        dst = k_sb[:, ot, jc * 512 : (jc + 1) * 512]
                        if jc % 8 < 5:
                            nc.vector.tensor_copy(out=dst, in_=ps)
                        else:
                            nc.scalar.activation(out=dst, in_=ps, func=ACT.Copy)

                for tb in range(N // 128):
                    ps = ps_big.tile([128, 512], f32, tag="psbig")
                    for ct in range(CT):
                        nc.tensor.matmul(
                            ps,
                            xb[:, ct, tb * 128 : (tb + 1) * 128],
                            wv_s[:, ct, :],
                            start=(ct == 0),
                            stop=(ct == CT - 1),
                        )
                    if tb % 8 < 5:
                        nc.vector.tensor_copy(out=vT_sb[:, tb, :], in_=ps)
                    else:
                        nc.scalar.activation(
                            out=vT_sb[:, tb, :], in_=ps, func=ACT.Copy
                        )

                # residual slice (full channels, my 1024 tokens); loaded late
                # so it doesn't compete with xb/weights for head DMA
                nc.sync.dma_start(
                    out=xres, in_=X.rearrange("a p t -> p a t")[:, :, 0:NB]
                )
                # fold the output bias into the residual tile: the epilogue
                # then needs a single add per query block
                for ot in range(CT):
                    nc.vector.tensor_scalar_add(
                        out=xres[:, ot, :],
                        in0=xres[:, ot, :],
                        scalar1=bias_o[:, ot, :],
                    )

            # ---- attention over 8 query blocks of 128 ----
            with (
                tc.tile_pool(name="loopp", bufs=2) as loopp,
                tc.tile_pool(name="sblk", bufs=2) as sblk,
                tc.tile_pool(name="ps_av", bufs=2, space="PSUM") as ps_av,
                tc.tile_pool(name="ps_o", bufs=2, space="PSUM") as ps_o,
            ):
                for ib in range(NB // 128):
                    i0 = ib * 128
                    p_sb = sblk.tile([128, N], bf16, tag="p_sb")
                    denp = loopp.tile([128, 4], f32, tag="denp")
                    for jc in range(N // 1024):
                        ps = ps_big.tile([128, 1024], f32, tag="psbig")
                        for half in range(2):
                            for c2 in range(2):
                                nc.tensor.matmul(
                                    ps[:, half * 512 : (half + 1) * 512],
                                    q_sb[:, 2 * c2 : 2 * c2 + 2, i0 : i0 + 128],
                                    k_sb[
                                        :,
                                        2 * c2 : 2 * c2 + 2,
                                        jc * 1024
                                        + half * 512 : jc * 1024
                                        + (half + 1) * 512,
                                    ],
                                    start=(c2 == 0),
                                    stop=(c2 == 1),
                                    perf_mode=DROW,
                                )
                        nc.scalar.activation(
                            out=p_sb[:, jc * 1024 : (jc + 1) * 1024],
                            in_=ps,
                            func=ACT.Exp,
                            accum_out=denp[:, jc : jc + 1],
                        )
                    den = loopp.tile([128, 1], f32, tag="den")
                    nc.vector.reduce_sum(out=den, in_=denp, axis=AX.X)
                    rden = loopp.tile([128, 1], f32, tag="rden")
                    nc.vector.reciprocal(out=rden, in_=den)

                    pT_sb = sblk.tile([128, N // 128, 128], bf16, tag="pT_sb")
                    nc.sync.dma_start_transpose(pT_sb, p_sb)

                    pav = ps_av.tile([128, C], f32, tag="pav")
                    for jb in range(N // 128):
                        nc.tensor.matmul(
                            pav,
                            pT_sb[:, jb, :],
                            vT_sb[:, jb, :],
                            start=(jb == 0),
                            stop=(jb == N // 128 - 1),
                        )
                    o2 = loopp.tile([128, C], bf16, tag="o2")
                    nc.vector.tensor_scalar_mul(out=o2, in0=pav, scalar1=rden)

                    ao = loopp.tile([128, CT, 128], bf16, tag="ao")
                    nc.sync.dma_start_transpose(ao, o2)

                    outf = loopp.tile([128, CT, 128], f32, tag="outf")
                    pso = ps_o.tile([128, CT, 128], f32, tag="pso")
                    for ot in range(CT):
                        for ct in range(CT):
                            nc.tensor.matmul(
                                pso[:, ot, :],
                                wo_b[:, ct, ot * 128 : (ot + 1) * 128],
                                ao[:, ct, :],
                                start=(ct == 0),
                                stop=(ct == CT - 1),
                            )
                    nc.vector.tensor_add(
                        out=outf, in0=pso, in1=xres[:, :, i0 : i0 + 128]
                    )
                    nc.sync.dma_start(out=OUT_p[:, :, i0 : i0 + 128], in_=outf)

    if split_waits:
        _split_sync_waits(nc)
    return nc


def _prep_in_maps(x, gn_gamma, gn_beta, wq, bq, wk, bk, wv, bv, wo, bo):
    import ml_dtypes

    f = np.float32
    bf = ml_dtypes.bfloat16
    xr = np.asarray(x, f).reshape(2, C, N)
    wq_t = np.ascontiguousarray(np.asarray(wq, f).T.astype(bf))
    wk_t = np.ascontiguousarray(np.asarray(wk, f).T.astype(bf))
    wv_t = np.ascontiguousarray(np.asarray(wv, f).T.astype(bf))
    wo_t = np.ascontiguousarray(np.asarray(wo, f).T.astype(bf))

    f8 = ml_dtypes.float8_e4m3  # matches mybir.dt.float8e4's layout

    def vec(v, dt=f):
        return np.ascontiguousarray(
            np.asarray(v, f).reshape(CT, 128).transpose(1, 0)[:, :, None].astype(dt)
        )

    gam = vec(gn_gamma)
    bet = vec(gn_beta)
    bq_s = vec(np.asarray(bq, f) * QKSCALE)
    bvv = vec(bv, bf)
    bov = vec(bo)

    cidx = np.arange(C)
    grp = cidx // 16  # (512,)
    gsum = np.zeros((128, CT, G), f)
    gbcast = np.zeros((G, CT, 128), f)
    for ct in range(CT):
        for cl in range(128):
            g = grp[ct * 128 + cl]
            gsum[cl, ct, g] = 1.0
            gbcast[g, ct, cl] = 1.0

    in_maps = []
    for core in range(NCORES):
        b, r = divmod(core, 4)
        xroll = np.ascontiguousarray(np.roll(xr[b], -r * NB, axis=1).reshape(CT, 128, N))
        in_maps.append(
            {
                "x": xroll,
                "x_bf": xroll.astype(bf),
                "x_f8": xroll.astype(f8),
                "wq_t": wq_t,
                "wk_t": wk_t,
                "wv_t": wv_t,
                "wo_t": wo_t,
                "gamma": gam,
                "beta": bet,
                "bq_s": bq_s,
                "bv": bvv,
                "bo": bov,
                "gsum": gsum,
                "gbcast": gbcast,
            }
        )
    return in_maps


def _assemble(results):
    out = np.empty((2, C, N), np.float32)
    for core in range(NCORES):
        b, r = divmod(core, 4)
        out[b][:, r * NB : (r + 1) * NB] = np.asarray(results[core]["out"]).reshape(
            C, NB
        )
    return out.reshape(2, C, 64, 64)


def _run(in_maps, trace=False, trace_kwargs=None):
    from concourse.bass_utils import run_bass_kernel_spmd

    if "nc" not in _cache:
        _cache["nc"] = build()
    kw = {}
    if trace:
        kw = {"trace": True, "trace_kwargs": trace_kwargs or {}}
    return run_bass_kernel_spmd(
        _cache["nc"], in_maps, core_ids=list(range(NCORES)), **kw
    )


def kernel(x, gn_gamma, gn_beta, wq, bq, wk, bk, wv, bv, wo, bo):
    in_maps = _prep_in_maps(x, gn_gamma, gn_beta, wq, bq, wk, bk, wv, bv, wo, bo)
    res = _run(in_maps, trace=False)
    return _assemble(res.results)
